# revision 26
# baseline (speedup 1.0000x reference)
"""Distributed GATv1 (2x GAT + SAGE + MLP head) for Trainium2, 8 NeuronCores.

v2 design (vs v1 baseline):
- g_full rows are 256 bf16 elems (512B) laid out [h(192) | al_s(3) | al_d(3) |
  pad], written by the dense phase via a zero-padded weight matrix, so one
  table serves the src-row gather and the dst-al_d path.
- Edge gathers use the dma_gather Q7 ucode (one call per (tile, half)) instead
  of per-edge-column indirect DMAs: ~1us Pool time per 1280 rows instead of
  ~1us per 128 rows.  Indices are int16, so nodes are split into two halves
  (gid < NG/2 and >=) and each tile's edge slots are grouped by half.
- Per-edge al_d comes from a one-hot routing matrix st_all[d,(j,e)] built once
  per tile (partition-broadcast matmuls + one is_equal) and Tg tiny matmuls
  against the tile's local al_d rows.
- All per-edge-column element-wise work is batched into a handful of whole-
  tile DVE/ACT ops; everything on the edge path is bf16 (PSUM accums in f32).
"""

import numpy as np

N = 50000
E = 800000
IN_C = 128
HID = 64
HEADS = 3
OUT_C = 16
C = HEADS * HID          # 192
NCORES = 8
P = 128
RW = 256                 # padded row width (bf16) = 512B
ALS0 = C                 # 192: al_s columns
ALD0 = C + HEADS         # 195: al_d columns


def _ceil(a, b):
    return -(-a // b)


def _pack_bins(deg_lo, deg_hi, nbins):
    """Greedy 2D-balanced binning: assign n=nbins*128 nodes to bins of 128
    slots, minimizing the max per-bin edge count for the lo- and hi-half
    source groups separately. Returns (bin_of, slot_of)."""
    n = len(deg_lo)
    assert n == nbins * P
    order = np.argsort(-(deg_lo + deg_hi), kind="stable")
    load_lo = np.zeros(nbins, np.int64)
    load_hi = np.zeros(nbins, np.int64)
    bin_fill = np.zeros(nbins, np.int64)
    bin_of = np.zeros(n, np.int32)
    slot_of = np.zeros(n, np.int32)
    big = np.int64(1 << 60)
    for l in order:
        cand = np.where(bin_fill < P,
                        np.maximum(load_lo + deg_lo[l], load_hi + deg_hi[l]),
                        big)
        b = int(np.argmin(cand))
        bin_of[l] = b
        slot_of[l] = bin_fill[b]
        bin_fill[b] += 1
        load_lo[b] += deg_lo[l]
        load_hi[b] += deg_hi[l]
    assert (bin_fill == P).all()
    return bin_of, slot_of


def _split_tiles(gsrc, dstperm, NT, HALF):
    """Group edges by dst tile, then by src half. Returns per-tile
    (lo_list, hi_list) of (src_gid_in_half, slot) arrays."""
    ebin = (dstperm // P).astype(np.int64)
    eslot = (dstperm % P).astype(np.int64)
    order = np.argsort(ebin, kind="stable")
    counts = np.bincount(ebin, minlength=NT)
    starts = np.zeros(NT + 1, np.int64)
    starts[1:] = np.cumsum(counts)
    out = []
    for t in range(NT):
        sel = order[starts[t]:starts[t + 1]]
        gs, sl = gsrc[sel], eslot[sel]
        m = gs < HALF
        out.append(((gs[m], sl[m]), (gs[~m] - HALF, sl[~m])))
    return out


def _wrap_idx(vals, T):
    """Index list -> dma_gather layout [128, 8*T] i16 (wrapped in 16
    partitions, replicated across the 8 Q7 core groups). Pads with 0."""
    n = T * P
    a = np.zeros(n, np.int16)
    a[:len(vals)] = vals.astype(np.int16)
    blk = a.reshape(n // 16, 16).T          # [16, 8*T]
    out = np.zeros((P, n // 16), np.int16)
    for g in range(8):
        out[g * 16:(g + 1) * 16] = blk
    return out


def _slot_grid(slots, T):
    """Slot list -> [128, T] f32 grid (edge i at [i%128, i//128]); pads -1."""
    n = T * P
    a = np.full(n, -1.0, np.float32)
    a[:len(slots)] = slots.astype(np.float32)
    return a.reshape(T, P).T.copy()         # [128, T]


def preprocess(x, edge_index, n_nodes, n_cores):
    """Host-side index preprocessing. Returns (cfg dict, per-core data, ggid)."""
    import ml_dtypes
    bf = ml_dtypes.bfloat16
    src = np.asarray(edge_index[0], np.int64)
    dst = np.asarray(edge_index[1], np.int64)
    NPC = n_nodes // n_cores
    NPpad = _ceil(NPC, P) * P
    NT = NPpad // P
    NG = n_cores * NPpad
    HALF = NG // 2

    x = np.asarray(x, np.float32)
    owner = dst // NPC
    # in-degree per node split by src half (cores 0..3 are the lo half of the
    # padded-global id space); +1 self-loop counts toward the node's own half
    src_is_lo = (src // NPC) < (n_cores // 2)
    deg_lo = np.bincount(dst[src_is_lo], minlength=n_nodes).astype(np.int64)
    deg_hi = np.bincount(dst[~src_is_lo], minlength=n_nodes).astype(np.int64)
    self_lo = (np.arange(n_nodes) // NPC) < (n_cores // 2)
    deg_lo += self_lo
    deg_hi += ~self_lo

    ggid = np.zeros(n_nodes, np.int64)
    pad_perm = []
    for k in range(n_cores):
        lo, hi = k * NPC, (k + 1) * NPC
        npd = NPpad - NPC
        dlo = np.concatenate([deg_lo[lo:hi], np.ones(npd, np.int64)])
        dhi = np.concatenate([deg_hi[lo:hi], np.zeros(npd, np.int64)])
        b, s = _pack_bins(dlo, dhi, NT)
        ggid[lo:hi] = k * NPpad + b[:NPC].astype(np.int64) * P + s[:NPC]
        pad_perm.append(b[NPC:].astype(np.int64) * P + s[NPC:])

    # per-core per-tile edge groups
    gat_tiles, sage_tiles = [], []
    for k in range(n_cores):
        m = owner == k
        es, ed = src[m], dst[m]
        sl_nodes = np.arange(k * NPC, (k + 1) * NPC, dtype=np.int64)
        ges = np.concatenate([es, sl_nodes])
        ged = np.concatenate([ed, sl_nodes])
        g_src = ggid[ges]
        g_dst = ggid[ged] - k * NPpad
        if len(pad_perm[k]):
            g_src = np.concatenate(
                [g_src, np.full(len(pad_perm[k]), ggid[0], np.int64)])
            g_dst = np.concatenate([g_dst, pad_perm[k]])
        gat_tiles.append(_split_tiles(g_src, g_dst, NT, HALF))
        s_src = ggid[es]
        s_dst = ggid[ed] - k * NPpad
        sage_tiles.append(_split_tiles(s_src, s_dst, NT, HALF))

    def _tmax_per_tile(tiles_all, gi):
        return [max(1, max(_ceil(len(tiles_all[k][t][gi][0]), P)
                           for k in range(n_cores))) for t in range(NT)]

    TGL = _tmax_per_tile(gat_tiles, 0)
    TGH = _tmax_per_tile(gat_tiles, 1)
    TSL = _tmax_per_tile(sage_tiles, 0)
    TSH = _tmax_per_tile(sage_tiles, 1)
    TG = [a + b for a, b in zip(TGL, TGH)]
    TS = [a + b for a, b in zip(TSL, TSH)]
    TGmax, TSmax = max(TG), max(TS)

    cores = []
    for k in range(n_cores):
        # per-tile packed meta: gat [idx_lo|idx_hi|slots], sage [dg|idx|slots]
        metag = np.zeros((NT, P, 9 * TGmax), np.int16)
        slrg = np.zeros((NT, 1, TGmax * P), np.float32)
        metas = np.zeros((NT, P, 2 + 9 * TSmax), np.int16)
        m = owner == k
        s_dst = ggid[dst[m]] - k * NPpad
        degs = np.bincount(s_dst, minlength=NPpad).astype(np.float32)
        deginv = (1.0 / np.maximum(degs, 1.0)).reshape(NT, P)
        for t in range(NT):
            tgl, tgh, tg = TGL[t], TGH[t], TG[t]
            (lg, ls_), (hg, hs) = gat_tiles[k][t]
            metag[t, :, :8 * tgl] = _wrap_idx(lg, tgl)
            metag[t, :, 8 * tgl:8 * tg] = _wrap_idx(hg, tgh)
            slot = np.concatenate(
                [_slot_grid(ls_, tgl), _slot_grid(hs, tgh)], axis=1)
            metag[t, :, 8 * tg:9 * tg] = slot.astype(bf).view(np.int16)
            slrg[t, 0, :tg * P] = slot.T.reshape(-1)
            tsl, tsh, ts_ = TSL[t], TSH[t], TS[t]
            (lg, ls_), (hg, hs) = sage_tiles[k][t]
            metas[t, :, 0:2] = deginv[t].astype(np.float32)[:, None].view(
                np.int16)
            metas[t, :, 2:2 + 8 * tsl] = _wrap_idx(lg, tsl)
            metas[t, :, 2 + 8 * tsl:2 + 8 * ts_] = _wrap_idx(hg, tsh)
            slot = np.concatenate(
                [_slot_grid(ls_, tsl), _slot_grid(hs, tsh)], axis=1)
            metas[t, :, 2 + 8 * ts_:2 + 9 * ts_] = slot.astype(bf).view(np.int16)
        # x shard in permuted order, pretransposed, bf16
        x_sh = np.zeros((NPpad, x.shape[1]), np.float32)
        lperm = ggid[k * NPC:(k + 1) * NPC] - k * NPpad
        x_sh[lperm] = x[k * NPC:(k + 1) * NPC]
        cores.append(dict(
            xT=np.ascontiguousarray(x_sh.T).astype(bf),
            metag=np.ascontiguousarray(metag),
            slrg=np.ascontiguousarray(slrg).astype(bf),
            metas=np.ascontiguousarray(metas),
        ))

    cfg = dict(n_cores=n_cores, NPC=NPC, NP=NPpad, NT=NT, NG=NG, HALF=HALF,
               TGL=TGL, TGH=TGH, TSL=TSL, TSH=TSH,
               TGmax=TGmax, TSmax=TSmax, Fin=x.shape[1])
    return cfg, cores, ggid


def fold_weights(W1, a1s, a1d, b1, W2, a2s, a2d, b2, Wl, bl, Wr, M1, mb1, M2, mb2):
    """Host-side weight folding -> replicated device weight arrays (bf16)."""
    import ml_dtypes
    bf = ml_dtypes.bfloat16
    f = lambda a: np.asarray(a, np.float32)
    W1, a1s, a1d, b1 = f(W1), f(a1s), f(a1d), f(b1)
    W2, a2s, a2d, b2 = f(W2), f(a2s), f(a2d), f(b2)
    Wl, bl, Wr, M1, mb1, M2, mb2 = f(Wl), f(bl), f(Wr), f(M1), f(mb1), f(M2), f(mb2)

    def bd(a):  # [HEADS, HID] -> block diag [C, HEADS]
        out = np.zeros((C, HEADS), np.float32)
        for h in range(HEADS):
            out[h * HID:(h + 1) * HID, h] = a[h]
        return out

    def pad256(w):  # [K, 198] -> [K, 256]
        out = np.zeros((w.shape[0], RW), np.float32)
        out[:, :w.shape[1]] = w
        return out

    # feature permutation: h-major (h*64+c) -> channel-major (c*3+h), so that
    # per-head broadcasts on device have stride-1 innermost (DVE 2x mode)
    perm = np.array([(k % HEADS) * HID + k // HEADS for k in range(C)])

    w1cat = pad256(np.concatenate([W1[:, perm], W1 @ bd(a1s), W1 @ bd(a1d)], 1))
    w2cat = pad256(np.concatenate([W2[:, perm], W2 @ bd(a2s), W2 @ bd(a2d)], 1))
    w2cat[:C] = w2cat[perm]          # rows follow f2's (c,h) order
    wlmm = (Wl @ M1 @ M2)[perm]
    wrmm = (Wr @ M1 @ M2)[perm]
    cvec = bl @ M1 @ M2 + mb1 @ M2 + mb2
    return dict(
        w1cat=np.ascontiguousarray(w1cat).astype(bf),
        w2cat=np.ascontiguousarray(w2cat).astype(bf),
        wlmm=np.ascontiguousarray(wlmm).astype(bf),
        wrmm=np.ascontiguousarray(wrmm).astype(bf),
        brep1=np.ascontiguousarray(np.tile(b1[None, perm], (P, 1))).astype(bf),
        brep2=np.ascontiguousarray(np.tile(b2[None, perm], (P, 1))).astype(bf),
        crep=np.ascontiguousarray(np.tile(cvec[None, :], (P, 1))),
    )


def build_program(cfg):
    """Build the Bass/Tile program (SPMD, identical across cores)."""
    import concourse.bass as bass
    import concourse.bacc as bacc
    import concourse.mybir as mybir
    import concourse.tile as tile
    from concourse.masks import make_identity
    from concourse import library_config

    n_cores = cfg["n_cores"]
    NP_, NT_ = cfg["NP"], cfg["NT"]
    NG, HALF = cfg["NG"], cfg["HALF"]
    TGL, TGH, TSL, TSH = cfg["TGL"], cfg["TGH"], cfg["TSL"], cfg["TSH"]
    TG = [a + b for a, b in zip(TGL, TGH)]
    TS = [a + b for a, b in zip(TSL, TSH)]
    TGmax, TSmax = cfg["TGmax"], cfg["TSmax"]
    Fin = cfg["Fin"]
    f32 = mybir.dt.float32
    bf16 = mybir.dt.bfloat16
    i16 = mybir.dt.int16
    i32 = mybir.dt.int32
    A = mybir.AluOpType
    ACT = mybir.ActivationFunctionType

    nq = int(cfg.get("nq", 4))
    gchunk = int(cfg.get("gchunk", 2))
    nc = bacc.Bacc("TRN2", target_bir_lowering=False, num_devices=n_cores,
                   num_swdge_queues=nq,
                   dynamic_dma_scratch_size=int(cfg.get("scratch", 32768)))
    _gq = [0]
    def _next_q():
        _gq[0] = (_gq[0] + 1) % nq
        return _gq[0]

    # I/O
    xT_in = nc.dram_tensor("xT", [Fin, NP_], bf16, kind="ExternalInput")
    w1cat = nc.dram_tensor("w1cat", [Fin, RW], bf16, kind="ExternalInput")
    w2cat = nc.dram_tensor("w2cat", [C, RW], bf16, kind="ExternalInput")
    wlmm = nc.dram_tensor("wlmm", [C, OUT_C], bf16, kind="ExternalInput")
    wrmm = nc.dram_tensor("wrmm", [C, OUT_C], bf16, kind="ExternalInput")
    brep1 = nc.dram_tensor("brep1", [P, C], bf16, kind="ExternalInput")
    brep2 = nc.dram_tensor("brep2", [P, C], bf16, kind="ExternalInput")
    crep = nc.dram_tensor("crep", [P, OUT_C], f32, kind="ExternalInput")
    metag = nc.dram_tensor("metag", [NT_, P, 9 * TGmax], i16,
                           kind="ExternalInput")
    slrg = nc.dram_tensor("slrg", [NT_, 1, TGmax * P], bf16,
                          kind="ExternalInput")
    metas = nc.dram_tensor("metas", [NT_, P, 2 + 9 * TSmax], i16,
                           kind="ExternalInput")
    out_sh = nc.dram_tensor("out_sh", [NP_, OUT_C], f32, kind="ExternalOutput")

    g1_loc = nc.dram_tensor("g1_loc", [NP_, RW], bf16, kind="Internal")
    f2 = nc.dram_tensor("f2", [NP_, C], bf16, kind="Internal")
    g2_loc = nc.dram_tensor("g2_loc", [NP_, RW], bf16, kind="Internal")
    f3_loc = nc.dram_tensor("f3_loc", [NP_, RW], bf16, kind="Internal")
    if n_cores > 1:
        aspace = "Shared" if n_cores > 4 else "Local"
        g1_full = nc.dram_tensor("g1_full", [NG, RW], bf16, kind="Internal",
                                 addr_space=aspace)
        g2_full = nc.dram_tensor("g2_full", [NG, RW], bf16, kind="Internal",
                                 addr_space=aspace)
        f3_full = nc.dram_tensor("f3_full", [NG, RW], bf16, kind="Internal",
                                 addr_space=aspace)
    else:
        g1_full, g2_full, f3_full = g1_loc, g2_loc, f3_loc


    with tile.TileContext(nc) as tc:
        import contextlib
        ctx = contextlib.ExitStack()
        with ctx:
            cpool = ctx.enter_context(tc.tile_pool(name="const", bufs=1))
            dpool = ctx.enter_context(tc.tile_pool(name="dense", bufs=4))
            epool = ctx.enter_context(tc.tile_pool(name="edge", bufs=int(cfg.get("ebufs", 6))))
            spool = ctx.enter_context(tc.tile_pool(name="spool", bufs=4))
            accps = ctx.enter_context(tc.tile_pool(name="accps", bufs=2, space="PSUM"))
            brps = ctx.enter_context(tc.tile_pool(name="brps", bufs=2, space="PSUM"))
            tpps = ctx.enter_context(tc.tile_pool(name="tpps", bufs=2, space="PSUM"))
            smps = ctx.enter_context(tc.tile_pool(name="smps", bufs=2, space="PSUM"))

            nc.gpsimd.load_library(library_config.mlp)

            # constants
            iota_i = cpool.tile([P, P], i32)
            iota_b = cpool.tile([P, P], bf16)
            nc.gpsimd.iota(iota_i[:], pattern=[[1, P]], base=0, channel_multiplier=0)
            nc.vector.tensor_copy(iota_b[:], iota_i[:])
            ident_f = cpool.tile([P, P], f32)
            ident_b = cpool.tile([P, P], bf16)
            make_identity(nc, ident_f[:])
            nc.vector.tensor_copy(ident_b[:], ident_f[:])
            ipt_i = cpool.tile([P, 512], i32)
            ipt_b = cpool.tile([P, 512], bf16)
            nc.gpsimd.iota(ipt_i[:], pattern=[[0, 512]], base=0,
                           channel_multiplier=1)
            nc.vector.tensor_copy(ipt_b[:], ipt_i[:])
            ones_b = cpool.tile([1, P], bf16)
            nc.vector.memset(ones_b[:], 1.0)

            # resident weights
            w1_sb = cpool.tile([Fin, RW], bf16)
            nc.sync.dma_start(w1_sb[:], w1cat[:, :])
            w2a_sb = cpool.tile([P, RW], bf16)
            w2b_sb = cpool.tile([C - P, RW], bf16)
            nc.sync.dma_start(w2a_sb[:], w2cat[0:P, :])
            nc.sync.dma_start(w2b_sb[:], w2cat[P:C, :])
            wl_a = cpool.tile([P, OUT_C], bf16)
            wl_b = cpool.tile([C - P, OUT_C], bf16)
            wr_a = cpool.tile([P, OUT_C], bf16)
            wr_b = cpool.tile([C - P, OUT_C], bf16)
            nc.sync.dma_start(wl_a[:], wlmm[0:P, :])
            nc.sync.dma_start(wl_b[:], wlmm[P:C, :])
            nc.sync.dma_start(wr_a[:], wrmm[0:P, :])
            nc.sync.dma_start(wr_b[:], wrmm[P:C, :])
            b1_sb = cpool.tile([P, C], bf16)
            b2_sb = cpool.tile([P, C], bf16)
            c_sb = cpool.tile([P, OUT_C], f32)
            nc.sync.dma_start(b1_sb[:], brep1[:, :])
            nc.sync.dma_start(b2_sb[:], brep2[:, :])
            nc.sync.dma_start(c_sb[:], crep[:, :])

            def dense1(scope):
                XB = 4
                with nc.named_scope(scope):
                    for t0 in range(0, NT_, XB):
                        nb = min(XB, NT_ - t0)
                        xt = dpool.tile([P, XB * P], bf16, tag="xt")
                        nc.sync.dma_start(xt[:, 0:nb * P],
                                          xT_in[:, t0 * P:(t0 + nb) * P])
                        for i in range(nb):
                            t = t0 + i
                            gps = accps.tile([P, RW], f32, tag="acc")
                            nc.tensor.matmul(out=gps[:],
                                             lhsT=xt[:, i * P:(i + 1) * P],
                                             rhs=w1_sb[:], start=True, stop=True)
                            gsb = dpool.tile([P, RW], bf16, tag="gsb")
                            nc.scalar.copy(gsb[:], gps[:])
                            nc.sync.dma_start(g1_loc[t * P:(t + 1) * P, :], gsb[:])

            def dense2(scope):
                with nc.named_scope(scope):
                    for t in range(NT_):
                        fsb = dpool.tile([P, C], bf16, tag="fsb")
                        nc.sync.dma_start(fsb[:], f2[t * P:(t + 1) * P, :])
                        gps = accps.tile([P, RW], f32, tag="acc")
                        for bi, (wt, k0, kw) in enumerate(
                                [(w2a_sb, 0, P), (w2b_sb, P, C - P)]):
                            tp = tpps.tile([P, P], bf16, tag="tp")
                            nc.tensor.transpose(out=tp[:kw, :],
                                                in_=fsb[:, k0:k0 + kw],
                                                identity=ident_b[:])
                            ft = dpool.tile([P, P], bf16, tag="ft")
                            nc.scalar.copy(ft[:kw, :], tp[:kw, :])
                            nc.tensor.matmul(out=gps[:], lhsT=ft[:kw, :], rhs=wt[:],
                                             start=(bi == 0), stop=(bi == 1))
                        gsb = dpool.tile([P, RW], bf16, tag="gsb")
                        nc.scalar.copy(gsb[:], gps[:])
                        nc.sync.dma_start(g2_loc[t * P:(t + 1) * P, :], gsb[:])

            def allgather(loc, full, scope):
                with nc.named_scope(scope):
                    nc.gpsimd.collective_compute(
                        "AllGather", A.bypass,
                        replica_groups=[list(range(n_cores))],
                        ins=[loc[:, :]],
                        outs=[full[:, :]],
                    )

            def gat_edge(g_full_d, g_loc_d, b_sb, f_out, fo_width, scope):
                with nc.named_scope(scope):
                    for t in range(NT_):
                        tgl, tgh, tg = TGL[t], TGH[t], TG[t]
                        gw_ = 9 * tg
                        meta = epool.tile([P, gw_ + (gw_ % 2)], i16, tag="mi")
                        nc.sync.dma_start(meta[:, 0:gw_], metag[t, :, 0:gw_])
                        mi = meta[:, 0:8 * tg]
                        sl = meta[:, 8 * tg:9 * tg].bitcast(bf16)
                        slr = epool.tile([1, tg * P], bf16, tag="slr")
                        nc.sync.dma_start(slr[:], slrg[t, :, 0:tg * P])
                        aldt = epool.tile([P, HEADS], bf16, tag="aldt")
                        nc.sync.dma_start(
                            aldt[:], g_loc_d[t * P:(t + 1) * P, ALD0:ALD0 + HEADS])
                        G = epool.tile([P, tg, RW], bf16, tag="G")
                        if not cfg.get("no_gather"):
                            for h0, hw_, tbl in (
                                    (0, tgl, g_full_d[0:HALF, :]),
                                    (tgl, tgh, g_full_d[HALF:NG, :])):
                                for c0 in range(0, hw_, gchunk):
                                    ck = min(gchunk, hw_ - c0)
                                    j0 = h0 + c0
                                    nc.gpsimd.dma_gather(
                                        G[:, j0:j0 + ck, :], tbl,
                                        mi[:, 8 * j0:8 * (j0 + ck)],
                                        ck * P, ck * P, RW,
                                        queue_num=_next_q())
                        else:
                            nc.vector.memset(G[:, 0, :], 0.5)
                        # st_all[d, (j,e)] = (d == slot[e,j])
                        st_all = spool.tile([P, tg * P], bf16, tag="st")
                        brs = epool.tile([P, tg * P], bf16, tag="brs")
                        for c0 in range(0, tg * P, 512):
                            cw = min(512, tg * P - c0)
                            br = brps.tile([P, 512], f32, tag="br")
                            nc.tensor.matmul(out=br[:, 0:cw], lhsT=ones_b[:],
                                             rhs=slr[:, c0:c0 + cw],
                                             start=True, stop=True)
                            nc.scalar.copy(brs[:, c0:c0 + cw], br[:, 0:cw])
                        nc.vector.tensor_tensor(
                            out=st_all[:].rearrange("p (t e) -> p t e", e=P),
                            in0=ipt_b[:, 0:P].unsqueeze(1).broadcast_to(
                                [P, tg, P]),
                            in1=brs[:].rearrange("p (t e) -> p t e", e=P),
                            op=A.is_equal)
                        # alde[(e), (j,h)] via tg tiny matmuls
                        alde = smps.tile([P, tg * HEADS], f32, tag="sm")
                        for j in range(tg):
                            nc.tensor.matmul(
                                out=alde[:, j * HEADS:(j + 1) * HEADS],
                                lhsT=st_all[:, j * P:(j + 1) * P], rhs=aldt[:],
                                start=True, stop=True)
                        # attention weights w = exp(leaky_relu(al_s + al_d))
                        att = epool.tile([P, tg, HEADS], f32, tag="att")
                        nc.vector.tensor_tensor(
                            out=att[:], in0=G[:, :, ALS0:ALS0 + HEADS],
                            in1=alde[:].rearrange("p (t h) -> p t h", h=HEADS),
                            op=A.add)
                        e2 = epool.tile([P, tg, HEADS], f32, tag="e2")
                        nc.scalar.activation(e2[:], att[:], ACT.Exp, scale=0.2)
                        gw = G[:, :, ALS0:ALS0 + HEADS]
                        nc.scalar.activation(gw, att[:], ACT.Exp)
                        nc.vector.tensor_tensor(out=gw, in0=gw, in1=e2[:], op=A.max)
                        # scale messages: G[:, :, 0:C] *= w (per head)
                        gh = G[:, :, 0:C].rearrange("p t (c h) -> p t c h", h=HEADS)
                        gwb = G[:, :, ALS0:ALS0 + HEADS].unsqueeze(2).broadcast_to(
                            [P, tg, HID, HEADS])
                        nc.vector.tensor_tensor(out=gh, in0=gh, in1=gwb, op=A.mult)
                        # one-hot S and aggregation
                        S_all = spool.tile([P, tg, P], bf16, tag="S")
                        nc.vector.tensor_tensor(
                            out=S_all[:],
                            in0=iota_b[:].unsqueeze(1).broadcast_to([P, tg, P]),
                            in1=sl.unsqueeze(2).broadcast_to([P, tg, P]),
                            op=A.is_equal)
                        ps = accps.tile([P, ALD0], f32, tag="acc")
                        for j in range(tg):
                            nc.tensor.matmul(out=ps[:], lhsT=S_all[:, j, :],
                                             rhs=G[:, j, 0:ALD0],
                                             start=(j == 0), stop=(j == tg - 1))
                        zinv = epool.tile([P, HEADS], f32, tag="zinv")
                        nc.vector.reciprocal(zinv[:], ps[:, ALS0:ALD0])
                        osb = epool.tile([P, C], bf16, tag="osb")
                        nc.vector.tensor_tensor(
                            out=osb[:].rearrange("p (c h) -> p c h", h=HEADS),
                            in0=ps[:, 0:ALS0].rearrange("p (c h) -> p c h", h=HEADS),
                            in1=zinv[:].unsqueeze(1).broadcast_to([P, HID, HEADS]),
                            op=A.mult)
                        nc.vector.tensor_tensor(out=osb[:], in0=osb[:], in1=b_sb[:],
                                                op=A.add)
                        nc.scalar.activation(osb[:], osb[:], ACT.Relu)
                        nc.sync.dma_start(f_out[t * P:(t + 1) * P, 0:C], osb[:])

            def sage(scope):
                with nc.named_scope(scope):
                    for t in range(NT_):
                        tsl, tsh, ts_ = TSL[t], TSH[t], TS[t]
                        mw = 2 + 9 * ts_
                        meta = epool.tile([P, mw + (mw % 2)], i16, tag="mi")
                        nc.sync.dma_start(meta[:, 0:mw], metas[t, :, 0:mw])
                        dg = meta[:, 0:2].bitcast(f32)
                        mi = meta[:, 2:2 + 8 * ts_]
                        sl = meta[:, 2 + 8 * ts_:2 + 9 * ts_].bitcast(bf16)
                        G = epool.tile([P, ts_, RW], bf16, tag="G")
                        if not cfg.get("no_gather"):
                            for h0, hw_, tbl in (
                                    (0, tsl, f3_full[0:HALF, :]),
                                    (tsl, tsh, f3_full[HALF:NG, :])):
                                for c0 in range(0, hw_, gchunk):
                                    ck = min(gchunk, hw_ - c0)
                                    j0 = h0 + c0
                                    nc.gpsimd.dma_gather(
                                        G[:, j0:j0 + ck, :], tbl,
                                        mi[:, 8 * j0:8 * (j0 + ck)],
                                        ck * P, ck * P, RW,
                                        queue_num=_next_q())
                        else:
                            nc.vector.memset(G[:, 0, :], 0.5)
                        S_all = spool.tile([P, ts_, P], bf16, tag="S")
                        nc.vector.tensor_tensor(
                            out=S_all[:],
                            in0=iota_b[:].unsqueeze(1).broadcast_to([P, ts_, P]),
                            in1=sl.unsqueeze(2).broadcast_to([P, ts_, P]),
                            op=A.is_equal)
                        ps = accps.tile([P, C], f32, tag="acc")
                        for j in range(ts_):
                            nc.tensor.matmul(out=ps[:], lhsT=S_all[:, j, :],
                                             rhs=G[:, j, 0:C],
                                             start=(j == 0), stop=(j == ts_ - 1))
                        asb = epool.tile([P, C], bf16, tag="asb")
                        nc.vector.tensor_scalar(out=asb[:], in0=ps[:],
                                                scalar1=dg, scalar2=None,
                                                op0=A.mult)
                        h2sb = epool.tile([P, C], bf16, tag="h2sb")
                        nc.sync.dma_start(h2sb[:], f3_loc[t * P:(t + 1) * P, 0:C])
                        ops = smps.tile([P, OUT_C], f32, tag="sm")
                        blocks = [(asb, wl_a, 0, P), (asb, wl_b, P, C - P),
                                  (h2sb, wr_a, 0, P), (h2sb, wr_b, P, C - P)]
                        for bi, (xsb, wt, k0, kw) in enumerate(blocks):
                            tp = tpps.tile([P, P], bf16, tag="tp")
                            nc.tensor.transpose(out=tp[:kw, :], in_=xsb[:, k0:k0 + kw],
                                                identity=ident_b[:])
                            xt = epool.tile([P, P], bf16, tag="xt")
                            nc.scalar.copy(xt[:kw, :], tp[:kw, :])
                            nc.tensor.matmul(out=ops[:], lhsT=xt[:kw, :], rhs=wt[:],
                                             start=(bi == 0), stop=(bi == 3))
                        fin = epool.tile([P, OUT_C], f32, tag="fin")
                        nc.vector.tensor_tensor(out=fin[:], in0=ops[:], in1=c_sb[:],
                                                op=A.add)
                        nc.scalar.activation(fin[:], fin[:], ACT.Sigmoid)
                        nc.sync.dma_start(out_sh[t * P:(t + 1) * P, :], fin[:])

            # ---- program ----
            do_coll = n_cores > 1 and not cfg.get("no_coll")
            for _rep in range(int(cfg.get("reps", 1))):
                sfx = f"_r{_rep}" if _rep else ""
                dense1("dense1" + sfx)
                if do_coll:
                    allgather(g1_loc, g1_full, "ag1" + sfx)
                gat_edge(g1_full, g1_loc, b1_sb, f2, C, "edge1" + sfx)
                dense2("dense2" + sfx)
                if do_coll:
                    allgather(g2_loc, g2_full, "ag2" + sfx)
                gat_edge(g2_full, g2_loc, b2_sb, f3_loc, RW, "edge2" + sfx)
                if do_coll:
                    allgather(f3_loc, f3_full, "ag3" + sfx)
                sage("sage" + sfx)

    nc.compile()
    return nc


LAST_RESULTS = None  # BassKernelResults of the most recent kernel() call


def kernel(**inputs):
    global LAST_RESULTS
    import os
    x = np.asarray(inputs["x"], np.float32)
    edge_index = np.asarray(inputs["edge_index"])
    cfg, cores, ggid = preprocess(x, edge_index, N, NCORES)
    wts = fold_weights(
        inputs["W1"], inputs["a1s"], inputs["a1d"], inputs["b1"],
        inputs["W2"], inputs["a2s"], inputs["a2d"], inputs["b2"],
        inputs["Wl"], inputs["bl"], inputs["Wr"],
        inputs["M1"], inputs["mb1"], inputs["M2"], inputs["mb2"])
    nc = build_program(cfg)
    in_maps = [dict(core, **wts) for core in cores]

    from concourse import bass_utils
    res = bass_utils.run_bass_kernel_spmd(
        nc, in_maps, core_ids=list(range(NCORES)),
        trace=bool(int(os.environ.get("GAT_TRACE", "0"))))
    LAST_RESULTS = res
    NPp = cfg["NP"]
    out = np.zeros((N, OUT_C), np.float32)
    for k in range(NCORES):
        o = res.results[k]["out_sh"]  # [NP, OUT_C]
        lo, hi = k * cfg["NPC"], (k + 1) * cfg["NPC"]
        out[lo:hi] = o[ggid[lo:hi] - k * NPp]
    return out
